# revision 56
# baseline (speedup 1.0000x reference)
"""DisentangledSelfAttention (DeBERTa-style) Trainium2 Bass kernel.

Sharding: 8 cores = 4 batch-pairs x 2 head-halves.  Core c handles batches
(2*(c%4), 2*(c%4)+1) and heads [6*(c//4), 6*(c//4)+6).  Each core emits the
partial output sum over its 6 heads for its 2 batches; the host adds the two
head-half partials per batch (standard tensor-parallel gather).

Algebraic structure (same as the baseline kernel):
  rel[i, j] = j - i + 511 depends only on (j - i); for S=384 only rel rows
  128..894 (767 values) are used.  qp[i,p] = q[i].Kp[p] is bounced to DRAM
  [384x512] per (batch,head) and read back with row pitch 511 ("skew" read)
  which turns the per-row diagonal shift into a flat strided access; same for
  kq[j,p'] = k[j].Qp_rev[p'], which lands transposed and accumulates into the
  score PSUM via identity matmuls.  Scores are computed TRANSPOSED so softmax
  row sums come free from a ones-column in V and the exp output feeds AV
  directly (see scores_tile / av_block docstrings).

Schedule (the main change vs the baseline):
  The whole bounce pipeline (qp/kq matmuls + PSUM->SBUF copies + DRAM write +
  skew readback) for ALL 12 (batch, head) instances runs interleaved with the
  projection chains, instead of inside the attention phase.  This spreads the
  ~35us of bounce copies over the projection window where DVE/Act are
  otherwise idle, so the attention phase reduces to scores -> exp -> AV ->
  outproj, which is PE/Act-balanced.  The 12 skew readbacks land in
  persistent SBUF buffers.  All PSUM pools stay open for the whole kernel
  (2 chain + 2 bounce + 3 score + 1 AV banks = 8) because closing a pool
  emits an all-engine barrier.  av_blocks are emitted one pair late so their
  latency chain (exp -> AV -> reciprocal/scale -> transpose -> copy) overlaps
  the next pair's score tiles.  Startup DMAs are split so the first
  projection matmul starts ~2.2us in, and the output is stored as fp16
  (host upconverts) to halve store time.
"""

import os
import sys

import numpy as np

B, S, D, H = 8, 384, 768, 12
DH = D // H          # 64
MAX_POS = 512
NP = 767             # used relative positions (128..894)
SCALE = DH ** -0.5

NB = 2               # batches per core
NH = 6               # heads per core
DHALF = NH * DH      # 384 projection columns per core
NIT = S // 128       # 3 i/j tiles per batch
NKT = D // 128       # 6 contraction tiles over D
NOT = DHALF // 128   # 3 output tiles over the head half
NPP = 768            # positional axis padded to even
NW = 512             # per-i-tile window of the positional axis (511 used)
S2 = NB * S          # 768 tokens per core (2 batches)

_CACHE = {}


def _import_concourse():
    try:
        import concourse.bass  # noqa: F401
    except ImportError:
        for p in ("/opt/trn_rl_repo", "/root/.axon_site/_ro/trn_rl_repo"):
            if os.path.isdir(p) and p not in sys.path:
                sys.path.insert(0, p)
        import concourse.bass  # noqa: F401


def _build():
    """Build + finalize the per-core Bass program (identical on all cores)."""
    _import_concourse()
    import concourse.bass as bass
    import concourse.bacc as bacc
    import concourse.mybir as mybir
    import concourse.tile as tile
    from concourse.bass import ts
    from concourse.masks import make_identity
    from concourse.tile import add_dep_helper

    f32 = mybir.dt.float32
    f16 = mybir.dt.float16
    bf16 = mybir.dt.bfloat16
    ADD = mybir.AluOpType.add
    EXP = mybir.ActivationFunctionType.Exp

    nc = bacc.Bacc("TRN2", target_bir_lowering=False, debug=False)

    # ---------------- DRAM I/O ----------------
    xT = nc.dram_tensor("xT", [D, S2], bf16, kind="ExternalInput")
    wq = nc.dram_tensor("wq", [D, DHALF], bf16, kind="ExternalInput")
    wk = nc.dram_tensor("wk", [D, DHALF], bf16, kind="ExternalInput")
    wv = nc.dram_tensor("wv", [D, DHALF], bf16, kind="ExternalInput")
    wpk = nc.dram_tensor("wpk", [D, DHALF], bf16, kind="ExternalInput")
    wpq = nc.dram_tensor("wpq", [D, DHALF], bf16, kind="ExternalInput")
    wo = nc.dram_tensor("wo", [DHALF, D], bf16, kind="ExternalInput")
    relkT = nc.dram_tensor("relkT", [D, NPP], bf16, kind="ExternalInput")
    bq = nc.dram_tensor("bq", [DHALF], f32, kind="ExternalInput")
    bk = nc.dram_tensor("bk", [DHALF], f32, kind="ExternalInput")
    bo = nc.dram_tensor("bo", [D], f32, kind="ExternalInput")
    out = nc.dram_tensor("out", [S2, D], f16, kind="ExternalOutput")

    NI = NB * NH     # 12 (batch, head) instances
    # one fp8 bounce scratch per HEAD (both batches), qp/kq rows INTERLEAVED
    # at pitch 1024 so the pitch-1023 skew readback covers both halves (plus
    # a dead 128-col gap) with one contiguous 896B descriptor per (row,
    # tile): elem (b, t, p, w, c) at flat addr 393216*b + 131072*t + 1024*p
    # + 512*w + c.  Merging both batches into one write + one read halves
    # the SWDGE descriptor-generation serialization on the Pool engine.
    f8 = mybir.dt.float8e4
    NRW = 896        # skew-read row width: qp[0:384], gap, kq[512:896]
    bnc_dram = [
        nc.dram_tensor(f"bnc_scratch_{i}", [NIT, 128, 2, NW], f8)
        for i in range(NI)
    ]

    with tile.TileContext(nc) as tc:
        with (
            tc.tile_pool(name="const", bufs=1) as constp,
            tc.tile_pool(name="big", bufs=1) as bigp,
            tc.tile_pool(name="wpool", bufs=3) as wpool,
            tc.tile_pool(name="work", bufs=3) as workp,
            tc.tile_pool(name="small", bufs=4) as smallp,
            tc.tile_pool(name="psA", bufs=2, space="PSUM") as psA,
            tc.tile_pool(name="psBNC", bufs=2, space="PSUM") as psBNC,
            tc.tile_pool(name="psSC", bufs=3, space="PSUM") as psSC,
            tc.tile_pool(name="psAV", bufs=1, space="PSUM") as psAV,
        ):
            import concourse.bass as bass_mod

            qT_sb = bigp.tile([128, NOT, S2], bf16, tag="qT")
            kT_sb = bigp.tile([128, NOT, S2], bf16, tag="kT")
            # per-head 65-wide slots: column 64 of each slot holds 1.0 so
            # the AV matmul emits the softmax row sums as a free extra column
            v_sb = bigp.tile([128, NB * NIT, 65 * NH], bf16, tag="v")
            KpT_sb = bigp.tile([128, NOT, NPP], bf16, tag="KpT")
            QpTr_sb = bigp.tile([128, NOT, NPP], bf16, tag="QpTr")
            attnT_sb = bigp.tile([128, NOT, S2], bf16, tag="attnT")

            # persistent skew-read destinations, one per instance (fp8)
            cp_sb = [
                workp.tile([128, NIT, NRW], f8, tag="cp", bufs=NI,
                           name=f"cp{i}")
                for i in range(NI)
            ]

            # GPSIMD cannot access PSUM, so PSUM->SBUF copies are split
            # between the DVE and Activation engines only.
            cp_engs = [nc.vector.tensor_copy, nc.scalar.copy]
            _cp_ctr = [0]

            def next_eng():
                _cp_ctr[0] += 1
                return cp_engs[_cp_ctr[0] % 2]

            def inst_bh(i):
                return i // NH, i % NH

            ident = constp.tile([128, 128], bf16, tag="ident")
            bq_sb = constp.tile([128, NOT], f32, tag="bq")
            bk_sb = constp.tile([128, NOT], f32, tag="bk")
            bof = constp.tile([1, D], f32, tag="bo")
            bor = constp.tile([128, D], f32, tag="bor")

            xT_sb = bigp.tile([128, NKT, S2], bf16, tag="xT")
            rel_f = bigp.tile([128, NKT, NPP], bf16, tag="rel_f")
            rel_r = bigp.tile([128, NKT, NPP], bf16, tag="rel_r")

            # ---------- input DMAs (SP queue, arrival-ordered) -------------
            wq_sb = wpool.tile([128, NKT, DHALF], bf16, tag="w")
            wk_sb = wpool.tile([128, NKT, DHALF], bf16, tag="w")
            # tiny first chunks so the first q chain starts ~2.2us in
            nc.sync.dma_start(
                wq_sb[:, 0:1, :], wq[0:128].rearrange("(o p) c -> p o c", p=128)
            )
            nc.sync.dma_start(
                xT_sb[:, 0:1, 0:S],
                xT[0:128, 0:S].rearrange("(o p) c -> p o c", p=128),
            )
            nc.sync.dma_start(
                wq_sb[:, 1:6, :], wq[128:D].rearrange("(o p) c -> p o c", p=128)
            )
            nc.sync.dma_start(
                xT_sb[:, 1:6, 0:S],
                xT[128:D, 0:S].rearrange("(o p) c -> p o c", p=128),
            )
            nc.sync.dma_start(bq_sb[:], bq[:].rearrange("(o p) -> p o", p=128))
            nc.sync.dma_start(bk_sb[:], bk[:].rearrange("(o p) -> p o", p=128))
            nc.sync.dma_start(wk_sb[:], wk[:].rearrange("(o p) c -> p o c", p=128))
            nc.sync.dma_start(
                xT_sb[:, :, S:S2], xT[:, S:S2].rearrange("(o p) c -> p o c", p=128)
            )
            wpk_sb = wpool.tile([128, NKT, DHALF], bf16, tag="w")
            nc.sync.dma_start(
                wpk_sb[:], wpk[:].rearrange("(o p) c -> p o c", p=128)
            )
            wpq_sb = wpool.tile([128, NKT, DHALF], bf16, tag="w", name="wpq_sb")
            nc.sync.dma_start(
                wpq_sb[:], wpq[:].rearrange("(o p) c -> p o c", p=128)
            )
            nc.sync.dma_start(rel_f[:], relkT[:].rearrange("(o p) c -> p o c", p=128))
            wv_sb = wpool.tile([128, NKT, DHALF], bf16, tag="w", name="wv_sb")
            nc.sync.dma_start(wv_sb[:], wv[:].rearrange("(o p) c -> p o c", p=128))
            nc.sync.dma_start(bof[:], bo[:].unsqueeze(0))
            make_identity(nc, ident[:])
            nc.gpsimd.partition_broadcast(bor[:], bof[:])
            nc.gpsimd.memset(v_sb[:], 1.0)

            # reversed rel operand, built on-chip (DVE negative-step copy)
            nc.vector.tensor_copy(
                rel_r[:, :, NPP - 1 : NPP], rel_f[:, :, NPP - 1 : NPP]
            )
            for ko in range(NKT):
                fwd = rel_f[:, ko, 0 : NPP - 1]
                rev = bass_mod.AP(
                    fwd.tensor,
                    fwd.offset + (NPP - 2),
                    [[fwd.ap[0][0], 128], [-1, NPP - 1]],
                )
                nc.vector.tensor_copy(rel_r[:, ko, 0 : NPP - 1], rev)

            # reversed rel operand, built on-chip (DVE negative-step copy)
            nc.vector.tensor_copy(
                rel_r[:, :, NPP - 1 : NPP], rel_f[:, :, NPP - 1 : NPP]
            )
            for ko in range(NKT):
                fwd = rel_f[:, ko, 0 : NPP - 1]
                rev = bass_mod.AP(
                    fwd.tensor,
                    fwd.offset + (NPP - 2),
                    [[fwd.ap[0][0], 128], [-1, NPP - 1]],
                )
                nc.vector.tensor_copy(rel_r[:, ko, 0 : NPP - 1], rev)

            # ---------- chain emitters ------------------------------------
            def qk_chain(w_sb, bias_sb, dst, b, mo):
                """one q^T / k^T projection chain: [dout(part), i]."""
                ps_t = psA.tile([128, NW], f32, tag="ps", name="ps")
                for ko in range(NKT):
                    nc.tensor.matmul(
                        ps_t[:, :S],
                        w_sb[:, ko, ts(mo, 128)],
                        xT_sb[:, ko, b * S : (b + 1) * S],
                        start=(ko == 0),
                        stop=(ko == NKT - 1),
                    )
                # bias-add on Act (per-partition bias): keeps DVE free for
                # the bounce copies that dominate phase 1
                nc.scalar.activation(
                    dst[:, mo, b * S : (b + 1) * S],
                    ps_t[:, :S],
                    mybir.ActivationFunctionType.Identity,
                    bias=bias_sb[:, mo : mo + 1],
                )

            def v_chain(b, jt):
                """v : [j(part), dh] (no bias: v_bias+bv folded into bo)"""
                ps_t = psA.tile([128, NW], f32, tag="ps", name="ps")
                for ko in range(NKT):
                    nc.tensor.matmul(
                        ps_t[:, :DHALF],
                        xT_sb[:, ko, b * S + 128 * jt : b * S + 128 * (jt + 1)],
                        wv_sb[:, ko, :],
                        start=(ko == 0),
                        stop=(ko == NKT - 1),
                    )
                vrow = v_sb[:, NIT * b + jt, 0:64]
                vdst = bass.AP(
                    vrow.tensor, vrow.offset,
                    [[vrow.ap[0][0], 128], [65, NH], [1, 64]],
                )
                next_eng()(
                    vdst, ps_t[:, :DHALF].rearrange("p (h c) -> p h c", h=NH)
                )

            def pos_chain(w_sb, rel_sb, dst, mo, ci):
                """one Kp^T / QpRev^T chain: [dout(part), p] 384-wide."""
                cs = 384 * ci
                ps_t = psA.tile([128, NW], f32, tag="ps", name="ps")
                for ko in range(NKT):
                    nc.tensor.matmul(
                        ps_t[:, :384],
                        w_sb[:, ko, ts(mo, 128)],
                        rel_sb[:, ko, cs : cs + 384],
                        start=(ko == 0),
                        stop=(ko == NKT - 1),
                    )
                next_eng()(dst[:, mo, cs : cs + 384], ps_t[:, :384])

            # ---------- bounce pipeline -----------------------------------
            bnc_sb = {}
            bnc_ps = [None]
            bnc_w = {}
            dbg = _CACHE.setdefault("debug", {"write": {}, "read": {}, "sc": {}})
            mm_q = []        # pending (i, which, it) bounce matmuls
            write_q = []     # instances written, awaiting skew read issue

            def bounce_mm(i, which, it):
                """one qp/kq windowed matmul + PSUM->SBUF copy; on the last
                tile of the head's second batch's kq half, issue the fused
                per-head DRAM write (both batches, both halves).

                For i-tile t only positional columns [256-128t, 768-128t)
                are ever read back, so each row tile computes a 512-wide
                window; bounce rows are stored with pitch 512.
                """
                b, h = inst_bh(i)
                hp, ho = 64 * (h % 2), h // 2
                if which == 0 and it == 0:
                    bnc_sb[i] = workp.tile(
                        [128, 2, NIT, NW], f8, tag="bounce", bufs=4,
                        name="bounce",
                    )
                sb = bnc_sb[i]
                w0 = 256 - 128 * it
                lhsT = (qT_sb if which == 0 else kT_sb)[
                    hp : hp + 64, ho, b * S + 128 * it : b * S + 128 * (it + 1)
                ]
                rhs = (KpT_sb if which == 0 else QpTr_sb)[
                    hp : hp + 64, ho, w0 : w0 + NW
                ]
                ps_t = psBNC.tile([128, NW], f32, tag="bnc", name="bnc")
                nc.tensor.matmul(ps_t[:], lhsT, rhs, start=True, stop=True)
                next_eng()(sb[:, which, it, :], ps_t[:])
                if which == 1 and it == NIT - 1:
                    # single fused DMA: both halves of instance i to DRAM.
                    # Pool-queue order keeps the write of instance n+1 ahead
                    # of the read of instance n so the read's input wait
                    # never idles the DMA engines.
                    bnc_w[i] = nc.gpsimd.dma_start(
                        bnc_dram[i].rearrange("t p w c -> p w t c"),
                        sb[:],
                    )
                    dbg["write"][i] = str(bnc_w[i].ins.name)
                    write_q.append(i)
                    if len(write_q) >= 2:
                        skew_read(write_q.pop(0))

            def skew_read(i):
                """fused skew readback for instance i into its cp slice.

                Partition stride 1023 vs write row pitch 1024 shifts each row
                by one column: row r of tile t starts at 131072*t + 1023*r +
                127 and spans 896 contiguous bytes covering the qp band
                [0:384) (= c2p[t][r, jf], col 127-r+jf of qp row r), the
                dead gap [384:512), and the kq band [512:896)
                (= p2cT[t][r, if]).
                """
                r1 = nc.gpsimd.dma_start(
                    cp_sb[i][:],
                    bass_mod.AP(
                        bnc_dram[i], 127,
                        [[1023, 128], [128 * 2 * NW, NIT], [1, NRW]],
                    ),
                )
                dbg["read"][i] = str(r1.ins.name)
                add_dep_helper(r1.ins, bnc_w[i].ins, reason="bounce rw")

            def push_group(mo):
                """queue both halves of both batches of the two heads in
                positional mo-group `mo`, instance-major so each instance's
                DRAM write fires right after its own six matmuls."""
                for h in (2 * mo, 2 * mo + 1):
                    for b in range(NB):
                        for which in range(2):
                            for it in range(NIT):
                                mm_q.append((b * NH + h, which, it))

            def pump(n):
                for _ in range(n):
                    if mm_q:
                        bounce_mm(*mm_q.pop(0))

            # ---------- attention emitters --------------------------------
            def scores_tile(i, jt, expT_sb):
                """transposed scores + exp for (instance, j-tile).

                scoresT[j, i] = c2c^T + p2c^T (straight identity-add from the
                kq skew read) + c2p^T (identity-matmul transposes of the qp
                skew read).  In this orientation the softmax row sums run
                over the PARTITION axis, so no accumulator readout is needed
                (the AV matmul's ones-column produces the sums instead) and
                the exp output feeds AV directly with no weight transpose.
                """
                b, h = inst_bh(i)
                hp, ho = 64 * (h % 2), h // 2
                cp = cp_sb[i]
                sc_ps = psSC.tile([128, NW], f32, tag="sc", name="sc")
                _m = nc.tensor.matmul(
                    sc_ps[:, :S],
                    kT_sb[hp : hp + 64, ho,
                          b * S + 128 * jt : b * S + 128 * (jt + 1)],
                    qT_sb[hp : hp + 64, ho, b * S : (b + 1) * S],
                    start=True,
                    stop=False,
                    skip_group_check=True,
                )
                if _m is not None and hasattr(_m, "ins"):
                    dbg["sc"][(i, jt)] = str(_m.ins.name)
                nc.tensor.matmul(
                    sc_ps[:, :S],
                    ident[:],
                    cp[:, jt, 512 : 512 + S],
                    start=False,
                    stop=False,
                    skip_group_check=True,
                )
                for t in range(NIT):
                    # out[jf, if] = sum_ip c2p[ip, 128*jt+jf] * I[ip, if]
                    nc.tensor.matmul(
                        sc_ps[:, ts(t, 128)],
                        cp[:, t, 128 * jt : 128 * (jt + 1)],
                        ident[:],
                        start=False,
                        stop=(t == NIT - 1),
                        skip_group_check=True,
                    )
                nc.scalar.activation(expT_sb[:, jt, :], sc_ps[:, :S], EXP)

            def av_block(i, expT_sb, av_ps, ofs, wt_ps, cp_eng=None):
                """AV + normalization + transpose back -> attnT.

                av_ps holds three 65-wide accumulation groups (one per
                i-tile); column 64 of each group is the softmax row sum
                (from v's ones column).  The per-partition reciprocal scale
                folds into the PSUM->SBUF copy, and the [i, dh] -> [dh, i]
                transposes are identity matmuls.  The pair's two instances
                are heads 2q/2q+1 of one batch, i.e. partition rows [0:64]
                and [64:128] of the SAME attnT mo-tile, so their transposes
                share wt_ps and the pair needs only ONE full-width
                PSUM->SBUF copy (issued here by the odd instance).
                """
                b, h = inst_bh(i)
                hp, ho = 64 * (h % 2), h // 2
                for it in range(NIT):
                    for jt in range(NIT):
                        nc.tensor.matmul(
                            av_ps[:, ofs + 65 * it : ofs + 65 * (it + 1)],
                            expT_sb[:, jt, ts(it, 128)],
                            v_sb[:, NIT * b + jt, 65 * h : 65 * (h + 1)],
                            start=(jt == 0),
                            stop=(jt == NIT - 1),
                            skip_group_check=True,
                        )
                attn_sb = workp.tile([128, NIT, 64], bf16, tag="attn", bufs=3)
                for it in range(NIT):
                    sinv = smallp.tile([128, 1], f32, tag="sinv")
                    nc.vector.reciprocal(
                        sinv[:],
                        av_ps[:, ofs + 65 * it + 64 : ofs + 65 * (it + 1)],
                    )
                    nc.vector.tensor_scalar_mul(
                        attn_sb[:, it, :],
                        av_ps[:, ofs + 65 * it : ofs + 65 * it + 64],
                        sinv[:],
                    )
                for it in range(NIT):
                    nc.tensor.matmul(
                        wt_ps[hp : hp + 64, ts(it, 128)],
                        attn_sb[:, it, :],
                        ident[:],
                        start=True,
                        stop=True,
                        skip_group_check=True,
                    )
                if hp:
                    (cp_eng or nc.vector.tensor_copy)(
                        attnT_sb[:, ho, b * S : (b + 1) * S],
                        wt_ps[:, :S],
                    )

            def outproj(b, it):
                """output projection for one 128-row tile of batch b."""
                o_sb = workp.tile([128, D], f16, tag="osb")
                for no in range(2):
                    ps_t = psA.tile([128, NW], f32, tag="ps", name="ps")
                    for ko in range(NOT):
                        nc.tensor.matmul(
                            ps_t[:, :384],
                            attnT_sb[:, ko,
                                     b * S + 128 * it : b * S + 128 * (it + 1)],
                            wo_sb[:, ko, ts(no, 384)],
                            start=(ko == 0),
                            stop=(ko == NOT - 1),
                        )
                    nc.vector.tensor_tensor(
                        o_sb[:, ts(no, 384)], ps_t[:, :384],
                        bor[:, ts(no, 384)], ADD,
                    )
                    # store each half as soon as its bias add lands: the
                    # second half's add overlaps the first half's store
                    nc.sync.dma_start(
                        out[b * S + 128 * it : b * S + 128 * (it + 1),
                            ts(no, 384)],
                        o_sb[:, ts(no, 384)],
                    )

            # ================= emission schedule ==========================
            # Chains are created in data-arrival order (psum-pool slots are
            # assigned in creation order, so an early-created chain whose
            # DMA lands late would block later chains through slot WARs);
            # each mo-group's bounce halves are queued right after the
            # chains they depend on, with ~5 bounce matmuls pumped per chain
            # so the whole DRAM round trip completes within the projection
            # phase.
            for b in range(NB):
                for mo in range(NOT):
                    qk_chain(wq_sb, bq_sb, qT_sb, b, mo)
            for b in range(NB):
                for mo in range(NOT):
                    qk_chain(wk_sb, bk_sb, kT_sb, b, mo)
            for ci in range(2):
                pos_chain(wpk_sb, rel_f, KpT_sb, 0, ci)
            for ci in range(2):
                pos_chain(wpq_sb, rel_r, QpTr_sb, 0, ci)
            push_group(0)
            fills = [
                lambda: pos_chain(wpk_sb, rel_f, KpT_sb, 1, 0),
                lambda: pos_chain(wpk_sb, rel_f, KpT_sb, 1, 1),
                lambda: pos_chain(wpq_sb, rel_r, QpTr_sb, 1, 0),
                lambda: pos_chain(wpq_sb, rel_r, QpTr_sb, 1, 1),
                lambda: push_group(1),
                lambda: v_chain(0, 0),
                lambda: v_chain(0, 1),
                lambda: v_chain(0, 2),
                lambda: pos_chain(wpk_sb, rel_f, KpT_sb, 2, 0),
                lambda: pos_chain(wpk_sb, rel_f, KpT_sb, 2, 1),
                lambda: pos_chain(wpq_sb, rel_r, QpTr_sb, 2, 0),
                lambda: pos_chain(wpq_sb, rel_r, QpTr_sb, 2, 1),
                lambda: push_group(2),
                lambda: v_chain(1, 0),
                lambda: v_chain(1, 1),
                lambda: v_chain(1, 2),
            ]
            for f in fills:
                f()
                pump(3)

            # prefetch Wo (reuses the wq weight slot; chain deps are over)
            wo_sb = wpool.tile([128, NOT, D], bf16, tag="w", name="wo_sb")
            nc.sync.dma_start(wo_sb[:], wo[:].rearrange("(o p) c -> p o c", p=128))

            # attention pairs; av_blocks are emitted ONE PAIR LATE so their
            # latency chain overlaps the next pair's score tiles, and the
            # leftover bounce matmuls of head-group 2 pump into pair 0/1
            exp_t = {}
            av_of = {}
            wt_of = {}

            def new_av(p):
                av_of[p] = psAV.tile([128, 2 * 3 * 65], f32, tag="avps",
                                     name="avps")

            NPAIR = NI // 2
            for p in range(NPAIR):
                i, j = 2 * p, 2 * p + 1
                exp_t[i] = workp.tile([128, NIT, S], bf16, tag="exp", bufs=4,
                                      name=f"exp{i}")
                exp_t[j] = workp.tile([128, NIT, S], bf16, tag="exp", bufs=4,
                                      name=f"exp{j}")
                for t in range(NIT):
                    scores_tile(i, t, exp_t[i])
                    pump(2)
                    scores_tile(j, t, exp_t[j])
                    pump(2)
                    if t == 0 and p >= 1:
                        new_av(p - 1)
                        # wt lives in the bounce pool (idle during
                        # attention) so the score pool's rotation holds
                        # only score tiles
                        wt_of[p - 1] = psBNC.tile([128, NW], f32,
                                                  tag="bnc", name="wt")
                        av_block(2 * (p - 1), exp_t[2 * (p - 1)],
                                 av_of[p - 1], 0, wt_of[p - 1])
                    if t == 1 and p >= 1:
                        av_block(2 * (p - 1) + 1, exp_t[2 * (p - 1) + 1],
                                 av_of[p - 1], 3 * 65, wt_of[p - 1])
                    if t == 2:
                        while write_q:
                            skew_read(write_q.pop(0))
                        if p == 4:
                            outproj(0, 0)
                        if p == 5:
                            outproj(0, 1)
            # drain: outproj 0/2 fills PE while the last exps finish, then
            # the final pair's AV and batch-1's output projection
            outproj(0, 2)
            new_av(NPAIR - 1)
            wt_of[NPAIR - 1] = psBNC.tile([128, NW], f32, tag="bnc",
                                          name="wt")
            av_block(2 * (NPAIR - 1), exp_t[2 * (NPAIR - 1)],
                     av_of[NPAIR - 1], 0, wt_of[NPAIR - 1],
                     cp_eng=nc.scalar.copy)
            av_block(2 * (NPAIR - 1) + 1, exp_t[2 * (NPAIR - 1) + 1],
                     av_of[NPAIR - 1], 3 * 65, wt_of[NPAIR - 1],
                     cp_eng=nc.scalar.copy)
            for it in range(NIT):
                outproj(1, it)

    nc.finalize()
    return nc
